# revision 1
# baseline (speedup 1.0000x reference)
"""CoxPH loss with Efron ties on 8 trn2 NeuronCores.

Math: with per-time tables over t in [0, 2048):
    s[t] = sum_{d_i=t} exp(hr_i)
    T[t] = sum_{d_i=t, e_i=1} exp(hr_i)
    n[t] = #{d_i=t, e_i=1}
    R[t] = suffix_sum(s)[t]
the Efron correction is
    corr = sum_t sum_{k=0}^{n_t-1} log(R_t - (k/n_t) T_t)
(each rank k in 0..n_t-1 appears exactly once among the tied events, so no
sort/rank machinery is needed), and
    loss = -(sum hr*e - corr) / (sum e + 1e-7).

Device plan (SPMD on 8 cores, data-parallel over samples):
  phase 1: per-core histogram via radix one-hots over t = dlo*32 + dhi
           (dhi = d & 31 compared 32-wide, dlo = d >> 5 compared 64-wide).
           Broadcast expansions of the digit/weight streams run on ScalarE
           and GpSimd (copies), so the VectorE is_equal/mult ops are dense
           bf16 (2x mode). Accumulating bf16 PE matmuls (FWL) bin into a
           PSUM [128, 64] table holding the s/T/n quadrants.
  AllReduce of the 3x2048 tables across the 8 cores.
  phase 2: R via triangular-ones matmul suffix sum; each core selects its
           own 2 time-columns via a per-core host-provided mask input, then
           runs a masked [128,1280] log grid per column with fused
           Ln+accumulate on ScalarE.
  Output per core: [128, 3] partials (corr, hr*e, n-sum); host does the final
  tiny reduction (the unshard step).
"""

import sys

sys.path.insert(0, "/opt/trn_rl_repo")

import numpy as np

import concourse.bacc as bacc
import concourse.bass as bass
import concourse.mybir as mybir
import concourse.tile as tile

NCORES = 8
N = 4_194_304
NPC = N // NCORES            # 524288 samples per core
P = 128
CTOT = NPC // P              # 4096 free-dim columns of samples
CS = 64                      # chunk size (columns per chunk)
NCHUNK = CTOT // CS
HI = 32                      # top 5 bits of t (d >> 6)
LO = 64                      # low 6 bits of t (d & 63)
NT = 2048                    # t = dhi*64 + dlo
FT = NT // P                 # 16 columns of 128 times
KMAX = 1280                  # static bound on max ties per time (mean 1024, sd 32)
COLS_PER_CORE = FT // NCORES  # 2

F32 = mybir.dt.float32
BF16 = mybir.dt.bfloat16
I32 = mybir.dt.int32
AL = mybir.AluOpType
AF = mybir.ActivationFunctionType

_COMPILED = None


def build():
    nc = bacc.Bacc("TRN2", target_bir_lowering=False, debug=False, num_devices=NCORES)

    hr_d = nc.dram_tensor("hr", [NPC], F32, kind="ExternalInput")
    dur_d = nc.dram_tensor("dur", [NPC], I32, kind="ExternalInput")
    evt_d = nc.dram_tensor("evt", [NPC], I32, kind="ExternalInput")
    iota32x_d = nc.dram_tensor("iota32x", [P, CS * HI], BF16, kind="ExternalInput")
    iota64x_d = nc.dram_tensor("iota64x", [P, CS * LO], BF16, kind="ExternalInput")
    iota64p1x_d = nc.dram_tensor("iota64p1x", [P, CS * LO], BF16, kind="ExternalInput")
    iotak_d = nc.dram_tensor("iotak", [P, KMAX], F32, kind="ExternalInput")
    tri128_d = nc.dram_tensor("tri128", [P, P], F32, kind="ExternalInput")  # [k,m]=k>=m
    tri16_d = nc.dram_tensor("tri16", [FT, FT], F32, kind="ExternalInput")  # [k,m]=k>m
    ones16_d = nc.dram_tensor("ones16", [FT, P], F32, kind="ExternalInput")
    # per-core column-selection masks: colsel[:, j*FT:(j+1)*FT] is a [P, FT]
    # 0/1 mask with a single 1-column marking this core's j-th grid column
    colsel_d = nc.dram_tensor(
        "colsel", [P, COLS_PER_CORE * FT], F32, kind="ExternalInput"
    )
    out_d = nc.dram_tensor("out", [P, 3], F32, kind="ExternalOutput")

    hr2 = hr_d.ap().rearrange("(p c) -> p c", p=P)
    dur2 = dur_d.ap().rearrange("(p c) -> p c", p=P)
    evt2 = evt_d.ap().rearrange("(p c) -> p c", p=P)

    with tile.TileContext(nc) as tc:
        with (
            tc.tile_pool(name="const", bufs=1) as constp,
            tc.tile_pool(name="data", bufs=1) as datap,
            tc.tile_pool(name="acc", bufs=1) as accp,
            tc.tile_pool(name="ps", bufs=1, space="PSUM") as psp,
            tc.tile_pool(name="dram", bufs=1, space="DRAM") as dramp,
        ):
            # ---- constants: dense repeated iota tiles, DMA'd from host ----
            iota32_x = constp.tile([P, CS, HI], BF16)
            nc.sync.dma_start(iota32_x[:], iota32x_d[:].rearrange("p (c j) -> p c j", j=HI))
            iota64_x = constp.tile([P, CS, LO], BF16)
            nc.sync.dma_start(iota64_x[:], iota64x_d[:].rearrange("p (c j) -> p c j", j=LO))
            iota64p1_x = constp.tile([P, CS, LO], BF16)
            nc.sync.dma_start(
                iota64p1_x[:], iota64p1x_d[:].rearrange("p (c j) -> p c j", j=LO)
            )

            # ---- full-width prep: digits, event-folded digits, weights ----
            dhi_b = datap.tile([P, CTOT], BF16)
            # bf16 [v|v] pair-packed streams in int32 containers: expansions
            # copy half the elements, then bitcast back to bf16 for the eqs
            pk_dlo = datap.tile([P, CTOT], I32)
            pk_dlo_e = datap.tile([P, CTOT], I32)
            pk_w = datap.tile([P, CTOT], I32)
            hre_acc = accp.tile([P, 1], F32)

            with tc.tile_pool(name="prep", bufs=1) as prepp:
                dlo_b = prepp.tile([P, CTOT], BF16, tag="dlo_b")
                dlo_e_b = prepp.tile([P, CTOT], BF16, tag="dlo_e_b")
                w_b = prepp.tile([P, CTOT], BF16, tag="w_b")
                hr_sb = prepp.tile([P, CTOT], F32)
                dur_sb = prepp.tile([P, CTOT], I32)
                evt_sb = prepp.tile([P, CTOT], I32)

                di = prepp.tile([P, CTOT], I32, tag="di")
                di2 = prepp.tile([P, CTOT], I32, tag="di2")
                e_b = prepp.tile([P, CTOT], BF16, tag="e_b")
                hre_secs = prepp.tile([P, 8], F32, tag="hre_secs")
                SEC = CTOT // 8
                for s8 in range(8):
                    ssl = slice(s8 * SEC, (s8 + 1) * SEC)
                    nc.sync.dma_start(dur_sb[:, ssl], dur2[:, ssl])
                    nc.sync.dma_start(hr_sb[:, ssl], hr2[:, ssl])
                    nc.sync.dma_start(evt_sb[:, ssl], evt2[:, ssl])
                    nc.vector.tensor_scalar(
                        di[:, ssl], dur_sb[:, ssl], 6, None, AL.logical_shift_right
                    )
                    nc.vector.tensor_copy(dhi_b[:, ssl], di[:, ssl])
                    nc.vector.tensor_scalar(
                        di2[:, ssl], dur_sb[:, ssl], 63, None, AL.bitwise_and
                    )
                    nc.vector.tensor_copy(dlo_b[:, ssl], di2[:, ssl])
                    nc.vector.tensor_copy(e_b[:, ssl], evt_sb[:, ssl])
                    # dlo1e = (dlo + 1) * e: 1..64 for events, 0 otherwise;
                    # compared against iota64p1 = 1..64 so censored rows match nothing
                    nc.vector.scalar_tensor_tensor(
                        dlo_e_b[:, ssl], dlo_b[:, ssl], 1.0, e_b[:, ssl],
                        AL.add, AL.mult,
                    )
                    nc.scalar.activation(w_b[:, ssl], hr_sb[:, ssl], AF.Exp)
                    scrap_f = prepp.tile([P, SEC], F32, tag="scrap_f")
                    nc.vector.scalar_tensor_tensor(
                        scrap_f[:], hr_sb[:, ssl], 1.0, e_b[:, ssl],
                        AL.mult, AL.mult,
                        accum_out=hre_secs[:, s8 : s8 + 1],
                    )
                    # pair-pack bf16 streams: pk = (bits << 16) | bits
                    for srcb, pk in ((dlo_b, pk_dlo), (dlo_e_b, pk_dlo_e), (w_b, pk_w)):
                        t32 = prepp.tile([P, SEC], I32, tag="t32")
                        nc.vector.tensor_copy(
                            t32[:], srcb[:, ssl].bitcast(mybir.dt.uint16)
                        )
                        s32 = prepp.tile([P, SEC], I32, tag="s32")
                        nc.vector.tensor_scalar(
                            s32[:], t32[:], 16, None, AL.logical_shift_left
                        )
                        nc.vector.tensor_tensor(pk[:, ssl], s32[:], t32[:], AL.bitwise_or)
                nc.vector.tensor_reduce(
                    hre_acc[:], hre_secs[:], mybir.AxisListType.X, AL.add
                )

            # ---- phase 1: histogram ----
            # two PSUM accumulators (even/odd tiles) to relax the single-bank
            # accumulation chain; summed after the loop
            table_ps = psp.tile([LO, P], F32)  # [w*hi|hi 64 rows] x [lo|lo_e 128 cols]
            table_ps2 = psp.tile([LO, P], F32)
            with (
                tc.tile_pool(name="xp", bufs=3) as xpp,
                tc.tile_pool(name="oh", bufs=2) as ohp,
                tc.tile_pool(name="grid", bufs=1) as gridp,
            ):
                for ch in range(NCHUNK):
                    c0 = ch * CS
                    sl = slice(c0, c0 + CS)
                    # pair-packed expansions on ScalarE: f32-container copies
                    # move 2 bf16 values per element (half the 1x element count)
                    dlo_x = xpp.tile([P, CS, LO // 2], F32, tag="dlo_x")
                    nc.scalar.copy(
                        dlo_x[:],
                        pk_dlo[:, sl].bitcast(F32).unsqueeze(2)
                        .broadcast_to([P, CS, LO // 2]),
                    )
                    dlo_e_x = xpp.tile([P, CS, LO // 2], F32, tag="dlo_e_x")
                    nc.scalar.copy(
                        dlo_e_x[:],
                        pk_dlo_e[:, sl].bitcast(F32).unsqueeze(2)
                        .broadcast_to([P, CS, LO // 2]),
                    )
                    w_x = xpp.tile([P, CS, HI // 2], F32, tag="w_x")
                    nc.scalar.copy(
                        w_x[:],
                        pk_w[:, sl].bitcast(F32).unsqueeze(2)
                        .broadcast_to([P, CS, HI // 2]),
                    )
                    dhi_x = xpp.tile([P, CS, HI], BF16, tag="dhi_x")
                    nc.scalar.copy(
                        dhi_x[:], dhi_b[:, sl].unsqueeze(2).broadcast_to([P, CS, HI])
                    )

                    # all one-hot builds dense bf16 on VectorE (2x mode)
                    lhs = ohp.tile([P, CS, P], BF16, tag="lhs")   # [0:64]=OHlo, [64:128]=OHlo_e
                    rhs = ohp.tile([P, CS, LO], BF16, tag="rhs")  # [0:32]=w*OHhi, [32:64]=OHhi
                    nc.vector.tensor_tensor(
                        lhs[:, :, 0:LO], dlo_x[:].bitcast(BF16), iota64_x[:],
                        AL.is_equal,
                    )
                    nc.vector.tensor_tensor(
                        lhs[:, :, LO : 2 * LO], dlo_e_x[:].bitcast(BF16),
                        iota64p1_x[:], AL.is_equal,
                    )
                    nc.vector.tensor_tensor(
                        rhs[:, :, HI : 2 * HI], dhi_x[:], iota32_x[:, :, 0:HI],
                        AL.is_equal,
                    )
                    nc.vector.tensor_tensor(
                        rhs[:, :, 0:HI],
                        rhs[:, :, HI : 2 * HI],
                        w_x[:].bitcast(BF16),
                        AL.mult,
                    )
                    for c in range(CS):
                        g = ch * CS + c
                        nc.tensor.matmul(
                            table_ps[:] if g % 2 == 0 else table_ps2[:],
                            rhs[:, c, :],
                            lhs[:, c, :],
                            start=(g < 2),
                            stop=(g >= CTOT - 2),
                        )

            # table quadrants (t = hi*64 + lo):
            #   s[hi, lo] = table[0:32, 0:64]    (w*hi rows x lo cols)
            #   T[hi, lo] = table[0:32, 64:128]  (w*hi rows x lo_e cols)
            #   n[hi, lo] = table[32:64, 64:128] (hi rows x lo_e cols)
            table_sb = accp.tile([LO, P], F32)
            nc.vector.tensor_copy(table_sb[:], table_ps2[:])
            nc.vector.tensor_tensor(table_sb[:], table_sb[:], table_ps[:], AL.add)

            ar_in = dramp.tile([3 * NT], F32)
            ar_out = dramp.tile([3 * NT], F32)
            nc.sync.dma_start(
                ar_in[:].rearrange("(a b) -> a b", a=3 * HI)[0:HI, :],
                table_sb[0:HI, 0:LO],
            )
            nc.sync.dma_start(
                ar_in[:].rearrange("(a b) -> a b", a=3 * HI)[HI : 2 * HI, :],
                table_sb[0:HI, LO:P],
            )
            nc.sync.dma_start(
                ar_in[:].rearrange("(a b) -> a b", a=3 * HI)[2 * HI : 3 * HI, :],
                table_sb[HI : 2 * HI, LO:P],
            )
            nc.gpsimd.collective_compute(
                "AllReduce",
                AL.add,
                replica_groups=[list(range(NCORES))],
                ins=[ar_in[:].opt()],
                outs=[ar_out[:].opt()],
            )

            # ---- phase 2 ----
            gridp2_cm = tc.tile_pool(name="grid2", bufs=1)
            gridp2 = gridp2_cm.__enter__()
            tri128 = constp.tile([P, P], F32)
            nc.sync.dma_start(tri128[:], tri128_d[:])
            tri16 = constp.tile([FT, FT], F32)
            nc.sync.dma_start(tri16[:], tri16_d[:])
            iotak = constp.tile([P, KMAX], F32)
            nc.sync.dma_start(iotak[:], iotak_d[:])
            ones16 = constp.tile([FT, P], F32)
            nc.sync.dma_start(ones16[:], ones16_d[:])
            colsel = constp.tile([P, COLS_PER_CORE * FT], F32)
            nc.sync.dma_start(colsel[:], colsel_d[:])

            # t = f*128 + p layouts
            s_a = accp.tile([P, FT], F32)
            nc.sync.dma_start(s_a[:], ar_out[0:NT].rearrange("(f p) -> p f", p=P))
            T_a = accp.tile([P, FT], F32)
            nc.sync.dma_start(T_a[:], ar_out[NT : 2 * NT].rearrange("(f p) -> p f", p=P))
            n_a = accp.tile([P, FT], F32)
            nc.sync.dma_start(
                n_a[:], ar_out[2 * NT : 3 * NT].rearrange("(f p) -> p f", p=P)
            )
            s_b = accp.tile([FT, P], F32)  # natural row-major [f, p] view
            nc.sync.dma_start(s_b[:], ar_out[0:NT].rearrange("(f p) -> f p", p=P))

            # R suffix sum: within-column suffix (tri128 @ s_a) plus the
            # cross-column offsets, both accumulated into one PSUM tile:
            #   offs[p, f] = sum_k ones[k, p] * (colsum[k] * [k > f])
            cs16 = accp.tile([FT, 1], F32)
            nc.vector.tensor_reduce(cs16[:], s_b[:], mybir.AxisListType.X, AL.add)
            csu = accp.tile([FT, FT], F32)
            nc.vector.tensor_scalar(csu[:], tri16[:], cs16[:, 0:1], None, AL.mult)
            rp_ps = psp.tile([P, FT], F32)
            nc.tensor.matmul(rp_ps[:], tri128[:], s_a[:], start=True, stop=False)
            nc.tensor.matmul(rp_ps[:], ones16[:], csu[:], start=False, stop=True)
            R = accp.tile([P, FT], F32)
            nc.vector.tensor_copy(R[:], rp_ps[:])

            # n is exactly integral (sums of exact 1.0s in f32); no rounding needed
            n_r = n_a
            n_s = accp.tile([P, FT], F32)
            nc.vector.tensor_scalar_max(n_s[:], n_r[:], 1.0)
            rec = accp.tile([P, FT], F32)
            nc.vector.reciprocal(rec[:], n_s[:])
            Tn = accp.tile([P, FT], F32)
            nc.vector.tensor_tensor(Tn[:], T_a[:], rec[:], AL.mult)
            negTn = accp.tile([P, FT], F32)
            nc.vector.tensor_scalar_mul(negTn[:], Tn[:], -1.0)

            nsum = accp.tile([P, 1], F32)
            nc.vector.tensor_reduce(nsum[:], n_r[:], mybir.AxisListType.X, AL.add)

            # grid over this core's columns, selected by the colsel mask:
            # my_x[j] = sum_f colsel[:, j*FT+f] * x[:, f]   (per-partition scalars)
            corr_cols = accp.tile([P, COLS_PER_CORE], F32)
            for j in range(COLS_PER_CORE):
                msl = slice(j * FT, (j + 1) * FT)
                my_negTn = accp.tile([P, 1], F32, tag="my_negTn")
                mscr = accp.tile([P, FT], F32, tag="mscr")
                nc.vector.tensor_tensor(mscr[:], negTn[:], colsel[:, msl], AL.mult)
                nc.vector.tensor_reduce(my_negTn[:], mscr[:], mybir.AxisListType.X, AL.add)
                my_R = accp.tile([P, 1], F32, tag="my_R")
                nc.vector.tensor_tensor(mscr[:], R[:], colsel[:, msl], AL.mult)
                nc.vector.tensor_reduce(my_R[:], mscr[:], mybir.AxisListType.X, AL.add)
                my_n = accp.tile([P, 1], F32, tag="my_n")
                nc.vector.tensor_tensor(mscr[:], n_r[:], colsel[:, msl], AL.mult)
                nc.vector.tensor_reduce(my_n[:], mscr[:], mybir.AxisListType.X, AL.add)

                arg = gridp2.tile([P, KMAX], F32, tag="arg")
                nc.vector.tensor_scalar(
                    arg[:], iotak[:], my_negTn[:, 0:1], my_R[:, 0:1], AL.mult, AL.add
                )
                mask = gridp2.tile([P, KMAX], F32, tag="mask")
                nc.vector.tensor_scalar(
                    mask[:], iotak[:], my_n[:, 0:1], None, AL.is_lt
                )
                margs = gridp2.tile([P, KMAX], F32, tag="margs")
                nc.vector.scalar_tensor_tensor(
                    margs[:], arg[:], 1.0, mask[:], AL.subtract, AL.mult
                )
                lscrap = gridp2.tile([P, KMAX], F32, tag="lscrap")
                nc.scalar.activation(
                    lscrap[:], margs[:], AF.Ln, bias=1.0,
                    accum_out=corr_cols[:, j : j + 1],
                )
            corr_acc = accp.tile([P, 1], F32)
            nc.vector.tensor_reduce(
                corr_acc[:], corr_cols[:], mybir.AxisListType.X, AL.add
            )

            # ---- output [128, 3] ----
            out_sb = accp.tile([P, 3], F32)
            nc.vector.tensor_copy(out_sb[:, 0:1], corr_acc[:])
            nc.vector.tensor_copy(out_sb[:, 1:2], hre_acc[:])
            nc.vector.tensor_copy(out_sb[:, 2:3], nsum[:])
            nc.sync.dma_start(out_d[:], out_sb[:])
            gridp2_cm.__exit__(None, None, None)

    nc.compile()
    return nc


def _consts():
    iota32 = np.tile(np.arange(HI), (P, 1)).astype(np.float32)
    iota64 = np.tile(np.arange(LO), (P, 1)).astype(np.float32)
    iotak = np.tile(np.arange(KMAX, dtype=np.float32), (P, 1))
    k = np.arange(P)
    tri128 = (k[:, None] >= k[None, :]).astype(np.float32)
    kf = np.arange(FT)
    tri16 = (kf[:, None] > kf[None, :]).astype(np.float32)
    return iota32, iota64, iotak, tri128, tri16


def kernel(hazard_ratio, durations, events):
    global _COMPILED
    import ml_dtypes
    from concourse.bass_utils import run_bass_kernel_spmd

    if _COMPILED is None:
        _COMPILED = build()
    nc = _COMPILED

    iota32, iota64, iotak, tri128, tri16 = _consts()
    iota32x = np.tile(np.arange(HI), (P, CS)).astype(ml_dtypes.bfloat16)
    iota64x = np.tile(np.arange(LO), (P, CS)).astype(ml_dtypes.bfloat16)
    iota64p1x = np.tile(np.arange(1, LO + 1), (P, CS)).astype(ml_dtypes.bfloat16)
    ones16 = np.ones((FT, P), dtype=np.float32)
    hr = np.ascontiguousarray(np.asarray(hazard_ratio, dtype=np.float32).reshape(-1))
    dur = np.ascontiguousarray(np.asarray(durations, dtype=np.int32).reshape(-1))
    evt = np.ascontiguousarray(np.asarray(events, dtype=np.int32).reshape(-1))

    in_maps = []
    for c in range(NCORES):
        sl = slice(c * NPC, (c + 1) * NPC)
        colsel = np.zeros((P, COLS_PER_CORE * FT), dtype=np.float32)
        for j in range(COLS_PER_CORE):
            colsel[:, j * FT + (c * COLS_PER_CORE + j)] = 1.0
        in_maps.append(
            {
                "hr": hr[sl],
                "dur": dur[sl],
                "evt": evt[sl],
                "iota32x": iota32x,
                "iota64x": iota64x,
                "iota64p1x": iota64p1x,
                "iotak": iotak,
                "tri128": tri128,
                "tri16": tri16,
                "ones16": ones16,
                "colsel": colsel,
            }
        )
    res = run_bass_kernel_spmd(nc, in_maps, list(range(NCORES)))

    outs = [res.results[c]["out"] for c in range(NCORES)]
    corr = np.float32(sum(o[:, 0].sum(dtype=np.float32) for o in outs))
    hre = np.float32(sum(o[:, 1].sum(dtype=np.float32) for o in outs))
    esum = outs[0][:, 2].sum(dtype=np.float32)
    loss = -(hre - corr) / (esum + np.float32(1e-7))
    return np.float32(loss).reshape(())



# revision 8
# speedup vs baseline: 1.4169x; 1.4169x over previous
"""CoxPH loss with Efron ties on 8 trn2 NeuronCores.

Math: with per-time tables over t in [0, 2048):
    s[t] = sum_{d_i=t} exp(hr_i)
    T[t] = sum_{d_i=t, e_i=1} exp(hr_i)
    n[t] = #{d_i=t, e_i=1}
    R[t] = suffix_sum(s)[t]
the Efron correction is
    corr = sum_t sum_{k=0}^{n_t-1} log(R_t - (k/n_t) T_t)
and loss = -(sum hr*e - corr) / (sum e + 1e-7).

Device plan (SPMD on 8 cores):
  Sharding: the loss is permutation-invariant over samples, so the host
  assigns samples to cores so every core receives exactly the same number
  of event samples, laid out events-first (column-major: device column c
  holds samples [128c, 128c+128)).  Per-column sample composition is then
  known at build time: pure-event columns, <=2 boundary (mixed) chunks,
  pure-censored columns.
  phase 1 (histogram via radix one-hots over t = dhi*64 + dlo):
    event columns:    stat = [w*OHhi | OHhi] (64), mov = OHlo (64)
                      -> psum quadrants T (=their s contribution) and n.
    censored columns: stat = [w*OHhi] (32), mov = OHlo (64) -> psum s.
    mixed chunk(s):   baseline scheme stat=[w*OHhi|OHhi] (64),
                      mov=[OHlo|OHlo_e] (128) -> s/T/n quadrants.
  This cuts the one-hot build from 192 to 128 VectorE elems/sample and
  nearly halves the ScalarE broadcast-expansion work.  Prep (digits, exp,
  bf16 pair-packing) is interleaved per 512-column section so it pipelines
  under the chunk loop instead of serializing in front of it.
  AllReduce of the 3x2048 tables across the 8 cores.
  phase 2: R via triangular-ones matmul suffix sum; each core selects its
  own 2 time-columns via a host-provided mask, then runs a masked
  [128,1280] log grid per column with fused Ln+accumulate on ScalarE.
  Output per core: [128, 3] partials (corr, hr*e, n-sum); host does the
  final tiny reduction (the unshard step).
"""

import sys

sys.path.insert(0, "/opt/trn_rl_repo")

import numpy as np

import concourse.bacc as bacc
import concourse.bass as bass
import concourse.mybir as mybir
import concourse.tile as tile

NCORES = 8
N = 4_194_304
NPC = N // NCORES            # 524288 samples per core
P = 128
CTOT = NPC // P              # 4096 free-dim columns of samples
CS = 64                      # chunk size (columns per chunk)
NCHUNK = CTOT // CS          # 64
SEC = 512                    # section size (columns) for interleaved prep
NSEC = CTOT // SEC           # 8
HI = 32                      # top 5 bits of t (d >> 6)
LO = 64                      # low 6 bits of t (d & 63)
NT = 2048                    # t = dhi*64 + dlo
FT = NT // P                 # 16 columns of 128 times
KMAX = 1280                  # static bound on max ties per time
COLS_PER_CORE = FT // NCORES  # 2

F32 = mybir.dt.float32
BF16 = mybir.dt.bfloat16
U16 = mybir.dt.uint16
I32 = mybir.dt.int32
AL = mybir.AluOpType
AF = mybir.ActivationFunctionType

_COMPILED = {}


def build(mc0, mc1, e_end, c_start):
    """mc0..mc1: chunk indices of the mixed region; e_end/c_start: sample
    boundaries (same on every core by construction)."""
    nc = bacc.Bacc("TRN2", target_bir_lowering=False, debug=False, num_devices=NCORES)

    hr_d = nc.dram_tensor("hr", [NPC], F32, kind="ExternalInput")
    dur_d = nc.dram_tensor("dur", [NPC], I32, kind="ExternalInput")
    evt_d = nc.dram_tensor("evt", [NPC], I32, kind="ExternalInput")
    iota32x_d = nc.dram_tensor("iota32x", [P, CS * HI], BF16, kind="ExternalInput")
    iota64x_d = nc.dram_tensor("iota64x", [P, CS * LO], BF16, kind="ExternalInput")
    iota64p1x_d = nc.dram_tensor("iota64p1x", [P, CS * LO], BF16, kind="ExternalInput")
    iotak_d = nc.dram_tensor("iotak", [P, KMAX], F32, kind="ExternalInput")
    tri128_d = nc.dram_tensor("tri128", [P, P], F32, kind="ExternalInput")  # [k,m]=k>=m
    tri16_d = nc.dram_tensor("tri16", [FT, FT], F32, kind="ExternalInput")  # [k,m]=k>m
    ones16_d = nc.dram_tensor("ones16", [FT, P], F32, kind="ExternalInput")
    colsel_d = nc.dram_tensor(
        "colsel", [P, COLS_PER_CORE * FT], F32, kind="ExternalInput"
    )
    out_d = nc.dram_tensor("out", [P, 3], F32, kind="ExternalOutput")

    hr2 = hr_d.ap().rearrange("(p c) -> p c", p=P)
    dur2 = dur_d.ap().rearrange("(p c) -> p c", p=P)
    evt2 = evt_d.ap().rearrange("(p c) -> p c", p=P)

    # section classification for the hr*e partial sums
    # sec covers samples [65536*s, 65536*(s+1))
    sec_kind = []
    for s in range(NSEC):
        lo_s, hi_s = SEC * P * s, SEC * P * (s + 1)
        if hi_s <= e_end:
            sec_kind.append("event")       # all events: sum hr
        elif lo_s >= c_start:
            sec_kind.append("cen")         # all censored: contributes 0
        else:
            sec_kind.append("mixed")       # needs evt data
    # chunk types
    chunk_kind = []
    for ch in range(NCHUNK):
        if ch < mc0:
            chunk_kind.append("event")
        elif ch < mc1:
            chunk_kind.append("mixed")
        else:
            chunk_kind.append("cen")
    n_e_cols = mc0 * CS
    n_m_cols = (mc1 - mc0) * CS
    n_c_cols = (NCHUNK - mc1) * CS

    with tile.TileContext(nc) as tc:
        with (
            tc.tile_pool(name="const", bufs=1) as constp,
            tc.tile_pool(name="acc", bufs=1) as accp,
            tc.tile_pool(name="ps", bufs=1, space="PSUM") as psp,
            tc.tile_pool(name="dram", bufs=1, space="DRAM") as dramp,
        ):
            # ---- constants: dense repeated iota tiles, DMA'd from host ----
            iota32_x = constp.tile([P, CS, HI], BF16)
            nc.sync.dma_start(iota32_x[:], iota32x_d[:].rearrange("p (c j) -> p c j", j=HI))
            iota64_x = constp.tile([P, CS, LO], BF16)
            nc.sync.dma_start(iota64_x[:], iota64x_d[:].rearrange("p (c j) -> p c j", j=LO))
            iota64p1_x = constp.tile([P, CS, LO], BF16)
            nc.sync.dma_start(
                iota64p1_x[:], iota64p1x_d[:].rearrange("p (c j) -> p c j", j=LO)
            )

            hre_secs = accp.tile([P, NSEC], F32)
            nc.vector.memset(hre_secs[:], 0.0)

            # PSUM accumulators (parity pairs relax same-bank accum chains)
            ps_e = psp.tile([LO, LO], F32)      # rows [w*OHhi|OHhi], cols OHlo
            ps_e2 = psp.tile([LO, LO], F32)
            ps_c = psp.tile([HI, LO], F32)      # rows w*OHhi, cols OHlo
            ps_c2 = psp.tile([HI, LO], F32)
            ps_m = psp.tile([LO, P], F32)       # rows [w*OHhi|OHhi], cols [OHlo|OHlo_e]

            g_e = g_c = g_m = 0  # per-stream matmul counters

            with (
                tc.tile_pool(name="sec", bufs=2) as secp,
                tc.tile_pool(name="xp", bufs=2) as xpp,
                tc.tile_pool(name="oh", bufs=2) as ohp,
                tc.tile_pool(name="mix", bufs=1) as mixp,
            ):
                for s in range(NSEC):
                    csl = slice(s * SEC, (s + 1) * SEC)
                    dur_sb = secp.tile([P, SEC], I32, tag="dur")
                    hr_sb = secp.tile([P, SEC], F32, tag="hr")
                    nc.sync.dma_start(dur_sb[:], dur2[:, csl])
                    nc.sync.dma_start(hr_sb[:], hr2[:, csl])

                    dlo_i = secp.tile([P, SEC], I32, tag="di")
                    dhi_i = secp.tile([P, SEC], I32, tag="di2")
                    dlo_b = secp.tile([P, SEC], BF16, tag="dlo_b")
                    dhi_b = secp.tile([P, SEC], BF16, tag="dhi_b")
                    w_b = secp.tile([P, SEC], BF16, tag="w_b")
                    nc.vector.tensor_scalar(dlo_i[:], dur_sb[:], 63, None, AL.bitwise_and)
                    nc.vector.tensor_copy(dlo_b[:], dlo_i[:])
                    nc.vector.tensor_scalar(
                        dhi_i[:], dur_sb[:], 6, None, AL.logical_shift_right
                    )
                    nc.vector.tensor_copy(dhi_b[:], dhi_i[:])
                    nc.scalar.activation(w_b[:], hr_sb[:], AF.Exp)

                    # hr*e partial for this section
                    if sec_kind[s] == "event":
                        nc.vector.tensor_reduce(
                            hre_secs[:, s : s + 1], hr_sb[:], mybir.AxisListType.X, AL.add
                        )
                    elif sec_kind[s] == "mixed":
                        evt_sb = secp.tile([P, SEC], I32, tag="evt")
                        nc.sync.dma_start(evt_sb[:], evt2[:, csl])
                        e_b = secp.tile([P, SEC], BF16, tag="e_b")
                        nc.vector.tensor_copy(e_b[:], evt_sb[:])
                        scrap_f = secp.tile([P, SEC], F32, tag="scrap_f")
                        nc.vector.scalar_tensor_tensor(
                            scrap_f[:], hr_sb[:], 1.0, e_b[:],
                            AL.mult, AL.mult,
                            accum_out=hre_secs[:, s : s + 1],
                        )

                    # pair-pack bf16 streams: pk = (bits << 16) | bits
                    packs = [("dlo", dlo_b), ("dhi", dhi_b), ("w", w_b)]
                    if sec_kind[s] == "mixed":
                        # dlo1e = (dlo + 1) * e: 1..64 for events, 0 otherwise
                        dlo_e_b = secp.tile([P, SEC], BF16, tag="dlo_e_b")
                        nc.vector.scalar_tensor_tensor(
                            dlo_e_b[:], dlo_b[:], 1.0, e_b[:], AL.add, AL.mult
                        )
                        packs.append(("dlo_e", dlo_e_b))
                    pk = {}
                    for nm, srcb in packs:
                        t32 = secp.tile([P, SEC], I32, tag="di")
                        nc.vector.tensor_copy(t32[:], srcb[:].bitcast(U16))
                        s32 = secp.tile([P, SEC], I32, tag="di2")
                        nc.vector.tensor_scalar(
                            s32[:], t32[:], 16, None, AL.logical_shift_left
                        )
                        pk_t = secp.tile([P, SEC], I32, tag=f"pk_{nm}")
                        nc.vector.tensor_tensor(pk_t[:], s32[:], t32[:], AL.bitwise_or)
                        pk[nm] = pk_t

                    for c8 in range(SEC // CS):
                        ch = s * (SEC // CS) + c8
                        kind = chunk_kind[ch]
                        sl = slice(c8 * CS, (c8 + 1) * CS)

                        # pair-packed expansions on ScalarE
                        dlo_x = xpp.tile([P, CS, LO // 2], F32, tag="dlo_x")
                        nc.scalar.copy(
                            dlo_x[:],
                            pk["dlo"][:, sl].bitcast(F32).unsqueeze(2)
                            .broadcast_to([P, CS, LO // 2]),
                        )
                        dhi_x = xpp.tile([P, CS, HI // 2], F32, tag="dhi_x")
                        nc.scalar.copy(
                            dhi_x[:],
                            pk["dhi"][:, sl].bitcast(F32).unsqueeze(2)
                            .broadcast_to([P, CS, HI // 2]),
                        )
                        w_x = xpp.tile([P, CS, HI // 2], F32, tag="w_x")
                        nc.scalar.copy(
                            w_x[:],
                            pk["w"][:, sl].bitcast(F32).unsqueeze(2)
                            .broadcast_to([P, CS, HI // 2]),
                        )

                        if kind == "mixed":
                            lhs = mixp.tile([P, CS, P], BF16, tag="lhs_m")
                            dlo_e_x = mixp.tile([P, CS, LO // 2], F32, tag="dlo_e_x")
                            nc.scalar.copy(
                                dlo_e_x[:],
                                pk["dlo_e"][:, sl].bitcast(F32).unsqueeze(2)
                                .broadcast_to([P, CS, LO // 2]),
                            )
                            nc.vector.tensor_tensor(
                                lhs[:, :, LO : 2 * LO], dlo_e_x[:].bitcast(BF16),
                                iota64p1_x[:], AL.is_equal,
                            )
                        else:
                            lhs = ohp.tile([P, CS, LO], BF16, tag="lhs")
                        nc.vector.tensor_tensor(
                            lhs[:, :, 0:LO], dlo_x[:].bitcast(BF16), iota64_x[:],
                            AL.is_equal,
                        )

                        rhs = ohp.tile([P, CS, LO], BF16, tag="rhs")
                        nc.vector.tensor_tensor(
                            rhs[:, :, HI : 2 * HI], dhi_x[:].bitcast(BF16),
                            iota32_x[:], AL.is_equal,
                        )
                        nc.vector.tensor_tensor(
                            rhs[:, :, 0:HI], rhs[:, :, HI : 2 * HI],
                            w_x[:].bitcast(BF16), AL.mult,
                        )
                        if kind == "event":
                            for c in range(CS):
                                nc.tensor.matmul(
                                    ps_e[:] if g_e % 2 == 0 else ps_e2[:],
                                    rhs[:, c, :],
                                    lhs[:, c, 0:LO],
                                    start=(g_e < 2),
                                    stop=(g_e >= n_e_cols - 2),
                                )
                                g_e += 1
                        elif kind == "cen":
                            for c in range(CS):
                                nc.tensor.matmul(
                                    ps_c[:] if g_c % 2 == 0 else ps_c2[:],
                                    rhs[:, c, 0:HI],
                                    lhs[:, c, 0:LO],
                                    start=(g_c < 2),
                                    stop=(g_c >= n_c_cols - 2),
                                )
                                g_c += 1
                        else:
                            for c in range(CS):
                                nc.tensor.matmul(
                                    ps_m[:],
                                    rhs[:, c, :],
                                    lhs[:, c, :],
                                    start=(g_m == 0),
                                    stop=(g_m == n_m_cols - 1),
                                )
                                g_m += 1

            hre_acc = accp.tile([P, 1], F32)
            nc.vector.tensor_reduce(
                hre_acc[:], hre_secs[:], mybir.AxisListType.X, AL.add
            )

            # ---- merge psums into the [96, 64] table (rows: s | T | n) ----
            if n_e_cols == 0:
                nc.vector.memset(ps_e[:], 0.0)
                nc.vector.memset(ps_e2[:], 0.0)
            if n_c_cols == 0:
                nc.vector.memset(ps_c[:], 0.0)
                nc.vector.memset(ps_c2[:], 0.0)
            # m_Tn rows 0:32 = T (partitions 0..31), rows 32:64 = n (32..63)
            m_Tn = accp.tile([LO, LO], F32)
            nc.vector.tensor_copy(m_Tn[:], ps_e[:])
            nc.vector.tensor_tensor(m_Tn[:], m_Tn[:], ps_e2[:], AL.add)
            if n_m_cols > 0:
                nc.vector.tensor_tensor(m_Tn[:], m_Tn[:], ps_m[:, LO:P], AL.add)
            # m_s (partitions 0..31) = ps_c + ps_c2 + T_evt (+ mix s-quadrant)
            m_s = accp.tile([HI, LO], F32)
            nc.vector.tensor_copy(m_s[:], ps_c[:])
            nc.vector.tensor_tensor(m_s[:], m_s[:], ps_c2[:], AL.add)
            nc.vector.tensor_tensor(m_s[:], m_s[:], ps_e[0:HI, :], AL.add)
            nc.vector.tensor_tensor(m_s[:], m_s[:], ps_e2[0:HI, :], AL.add)
            if n_m_cols > 0:
                nc.vector.tensor_tensor(m_s[:], m_s[:], ps_m[0:HI, 0:LO], AL.add)

            ar_in = dramp.tile([3 * NT], F32)
            ar_out = dramp.tile([3 * NT], F32)
            ar_v = ar_in[:].rearrange("(a b) -> a b", a=3 * HI)
            nc.sync.dma_start(ar_v[0:HI, :], m_s[:])
            nc.sync.dma_start(ar_v[HI : 3 * HI, :], m_Tn[:])
            nc.gpsimd.collective_compute(
                "AllReduce",
                AL.add,
                replica_groups=[list(range(NCORES))],
                ins=[ar_in[:].opt()],
                outs=[ar_out[:].opt()],
            )

            # ---- phase 2 ----
            gridp2_cm = tc.tile_pool(name="grid2", bufs=1)
            gridp2 = gridp2_cm.__enter__()
            tri128 = constp.tile([P, P], F32)
            nc.sync.dma_start(tri128[:], tri128_d[:])
            tri16 = constp.tile([FT, FT], F32)
            nc.sync.dma_start(tri16[:], tri16_d[:])
            iotak = constp.tile([P, KMAX], F32)
            nc.sync.dma_start(iotak[:], iotak_d[:])
            ones16 = constp.tile([FT, P], F32)
            nc.sync.dma_start(ones16[:], ones16_d[:])
            colsel = constp.tile([P, COLS_PER_CORE * FT], F32)
            nc.sync.dma_start(colsel[:], colsel_d[:])

            # t = f*128 + p layouts
            s_a = accp.tile([P, FT], F32)
            nc.sync.dma_start(s_a[:], ar_out[0:NT].rearrange("(f p) -> p f", p=P))
            T_a = accp.tile([P, FT], F32)
            nc.sync.dma_start(T_a[:], ar_out[NT : 2 * NT].rearrange("(f p) -> p f", p=P))
            n_a = accp.tile([P, FT], F32)
            nc.sync.dma_start(
                n_a[:], ar_out[2 * NT : 3 * NT].rearrange("(f p) -> p f", p=P)
            )
            s_b = accp.tile([FT, P], F32)  # natural row-major [f, p] view
            nc.sync.dma_start(s_b[:], ar_out[0:NT].rearrange("(f p) -> f p", p=P))

            # R suffix sum: within-column suffix (tri128 @ s_a) plus the
            # cross-column offsets, both accumulated into one PSUM tile
            cs16 = accp.tile([FT, 1], F32)
            nc.vector.tensor_reduce(cs16[:], s_b[:], mybir.AxisListType.X, AL.add)
            csu = accp.tile([FT, FT], F32)
            nc.vector.tensor_scalar(csu[:], tri16[:], cs16[:, 0:1], None, AL.mult)
            rp_ps = psp.tile([P, FT], F32)
            nc.tensor.matmul(rp_ps[:], tri128[:], s_a[:], start=True, stop=False)
            nc.tensor.matmul(rp_ps[:], ones16[:], csu[:], start=False, stop=True)
            R = accp.tile([P, FT], F32)
            nc.vector.tensor_copy(R[:], rp_ps[:])

            n_r = n_a
            n_s = accp.tile([P, FT], F32)
            nc.vector.tensor_scalar_max(n_s[:], n_r[:], 1.0)
            rec = accp.tile([P, FT], F32)
            nc.vector.reciprocal(rec[:], n_s[:])
            Tn = accp.tile([P, FT], F32)
            nc.vector.tensor_tensor(Tn[:], T_a[:], rec[:], AL.mult)
            negTn = accp.tile([P, FT], F32)
            nc.vector.tensor_scalar_mul(negTn[:], Tn[:], -1.0)

            nsum = accp.tile([P, 1], F32)
            nc.vector.tensor_reduce(nsum[:], n_r[:], mybir.AxisListType.X, AL.add)

            corr_cols = accp.tile([P, COLS_PER_CORE], F32)
            for j in range(COLS_PER_CORE):
                msl = slice(j * FT, (j + 1) * FT)
                my_negTn = accp.tile([P, 1], F32, tag="my_negTn")
                mscr = accp.tile([P, FT], F32, tag="mscr")
                nc.vector.tensor_tensor(mscr[:], negTn[:], colsel[:, msl], AL.mult)
                nc.vector.tensor_reduce(my_negTn[:], mscr[:], mybir.AxisListType.X, AL.add)
                my_R = accp.tile([P, 1], F32, tag="my_R")
                nc.vector.tensor_tensor(mscr[:], R[:], colsel[:, msl], AL.mult)
                nc.vector.tensor_reduce(my_R[:], mscr[:], mybir.AxisListType.X, AL.add)
                my_n = accp.tile([P, 1], F32, tag="my_n")
                nc.vector.tensor_tensor(mscr[:], n_r[:], colsel[:, msl], AL.mult)
                nc.vector.tensor_reduce(my_n[:], mscr[:], mybir.AxisListType.X, AL.add)

                arg = gridp2.tile([P, KMAX], F32, tag="arg")
                nc.vector.tensor_scalar(
                    arg[:], iotak[:], my_negTn[:, 0:1], my_R[:, 0:1], AL.mult, AL.add
                )
                mask = gridp2.tile([P, KMAX], F32, tag="mask")
                nc.vector.tensor_scalar(
                    mask[:], iotak[:], my_n[:, 0:1], None, AL.is_lt
                )
                margs = gridp2.tile([P, KMAX], F32, tag="margs")
                nc.vector.scalar_tensor_tensor(
                    margs[:], arg[:], 1.0, mask[:], AL.subtract, AL.mult
                )
                lscrap = gridp2.tile([P, KMAX], F32, tag="lscrap")
                nc.scalar.activation(
                    lscrap[:], margs[:], AF.Ln, bias=1.0,
                    accum_out=corr_cols[:, j : j + 1],
                )
            corr_acc = accp.tile([P, 1], F32)
            nc.vector.tensor_reduce(
                corr_acc[:], corr_cols[:], mybir.AxisListType.X, AL.add
            )

            # ---- output [128, 3] ----
            out_sb = accp.tile([P, 3], F32)
            nc.vector.tensor_copy(out_sb[:, 0:1], corr_acc[:])
            nc.vector.tensor_copy(out_sb[:, 1:2], hre_acc[:])
            nc.vector.tensor_copy(out_sb[:, 2:3], nsum[:])
            nc.sync.dma_start(out_d[:], out_sb[:])
            gridp2_cm.__exit__(None, None, None)

    nc.compile()
    return nc


def _consts():
    iota32 = np.tile(np.arange(HI), (P, 1)).astype(np.float32)
    iota64 = np.tile(np.arange(LO), (P, 1)).astype(np.float32)
    iotak = np.tile(np.arange(KMAX, dtype=np.float32), (P, 1))
    k = np.arange(P)
    tri128 = (k[:, None] >= k[None, :]).astype(np.float32)
    kf = np.arange(FT)
    tri16 = (kf[:, None] > kf[None, :]).astype(np.float32)
    return iota32, iota64, iotak, tri128, tri16


def _plan(evt_flat):
    """Equal-event sharding: per-core stream = [E events][p pool][C censored],
    identical E/p/C on every core.  Returns (perm[NCORES, NPC], mc0, mc1,
    e_end, c_start)."""
    ev_idx = np.flatnonzero(evt_flat)
    cen_idx = np.flatnonzero(evt_flat == 0)
    Etot, Ctot = ev_idx.size, cen_idx.size
    E, C = Etot // NCORES, Ctot // NCORES
    pool = np.concatenate([ev_idx[NCORES * E :], cen_idx[NCORES * C :]])
    p = pool.size // NCORES
    assert NCORES * E + NCORES * C + pool.size == N
    perms = []
    for c in range(NCORES):
        stream = np.concatenate(
            [
                ev_idx[c * E : (c + 1) * E],
                pool[c * p : (c + 1) * p],
                cen_idx[c * C : (c + 1) * C],
            ]
        )
        # column-major: device column k holds samples [128k, 128(k+1))
        perms.append(np.ascontiguousarray(stream.reshape(CTOT, P).T).reshape(-1))
    e_end = E
    c_start = E + p
    mc0 = e_end // (CS * P)
    mc1 = -(-c_start // (CS * P))  # ceil
    mc1 = max(mc1, mc0)
    return perms, mc0, mc1, e_end, c_start


def _in_maps(hazard_ratio, durations, events):
    import ml_dtypes

    hr = np.ascontiguousarray(np.asarray(hazard_ratio, dtype=np.float32).reshape(-1))
    dur = np.ascontiguousarray(np.asarray(durations, dtype=np.int32).reshape(-1))
    evt = np.ascontiguousarray(np.asarray(events, dtype=np.int32).reshape(-1))
    perms, mc0, mc1, e_end, c_start = _plan(evt)

    iota32, iota64, iotak, tri128, tri16 = _consts()
    iota32x = np.tile(np.arange(HI), (P, CS)).astype(ml_dtypes.bfloat16)
    iota64x = np.tile(np.arange(LO), (P, CS)).astype(ml_dtypes.bfloat16)
    iota64p1x = np.tile(np.arange(1, LO + 1), (P, CS)).astype(ml_dtypes.bfloat16)
    ones16 = np.ones((FT, P), dtype=np.float32)

    in_maps = []
    for c in range(NCORES):
        pi = perms[c]
        colsel = np.zeros((P, COLS_PER_CORE * FT), dtype=np.float32)
        for j in range(COLS_PER_CORE):
            colsel[:, j * FT + (c * COLS_PER_CORE + j)] = 1.0
        in_maps.append(
            {
                "hr": hr[pi],
                "dur": dur[pi],
                "evt": evt[pi],
                "iota32x": iota32x,
                "iota64x": iota64x,
                "iota64p1x": iota64p1x,
                "iotak": iotak,
                "tri128": tri128,
                "tri16": tri16,
                "ones16": ones16,
                "colsel": colsel,
            }
        )
    return in_maps, mc0, mc1, e_end, c_start


def _run(hazard_ratio, durations, events, trace=False, tmpdir=None):
    from concourse.bass_utils import run_bass_kernel_spmd

    in_maps, mc0, mc1, e_end, c_start = _in_maps(hazard_ratio, durations, events)
    key = (mc0, mc1, e_end, c_start)
    if key not in _COMPILED:
        _COMPILED.clear()
        _COMPILED[key] = build(*key)
    nc = _COMPILED[key]

    kw = {}
    if trace:
        kw = dict(trace=True, tmpdir=tmpdir)
    res = run_bass_kernel_spmd(nc, in_maps, list(range(NCORES)), **kw)

    outs = [res.results[c]["out"] for c in range(NCORES)]
    corr = np.float32(sum(o[:, 0].sum(dtype=np.float32) for o in outs))
    hre = np.float32(sum(o[:, 1].sum(dtype=np.float32) for o in outs))
    esum = outs[0][:, 2].sum(dtype=np.float32)
    loss = -(hre - corr) / (esum + np.float32(1e-7))
    return np.float32(loss).reshape(()), res


def kernel(hazard_ratio, durations, events):
    out, _ = _run(hazard_ratio, durations, events)
    return out
